# revision 26
# baseline (speedup 1.0000x reference)
"""3D Haar DWT (clean-mode subband stack) on 8 Trainium2 NeuronCores.

Problem (hardcoded): inputs (4, 128, 128, 128, 4) f32, A (128, 128) f32 Haar
analysis operator. Output (4, 64, 64, 64, 32) f32 = 8 subbands stacked on the
channel axis (LLL, LLH, LHL, LHH, HLL, HLH, HHL, HHH) x 4 channels.

Sharding: pure data parallel over (batch, d1-half): core k handles
b = k // 2, d1 range [64*(k%2), 64*(k%2)+64). The Haar transform is a 2-tap
non-overlapping filter (rows of A touch only columns 2i, 2i+1), so splitting
d1 on an even boundary requires no communication.

Per-core pipeline (slab x[64, 128, 128, 4] laid out as [d2 | d1, d3, c]):
  1. DMA in 2 MiB chunks (8 d1 slices), partitions = d2.
  2. d3 butterfly on DVE (free-axis strided add/sub).
  3. d2 transform as PE matmul (stationary 0.5*A^T, float32r fast path),
     with the d1 butterfly folded into PSUM accumulation (+/-0.5*A^T).
  4. PSUM -> SBUF evacuation doubles as the subband split (quadrant copies).
  5. DMA out per (s1, s3) block; host reassembles subband-major layout.

Scale bookkeeping: reference applies A three times (factor s = 1/sqrt(2) per
nonzero). Here the d3/d1 butterflies apply +/-1 and the matmul applies
0.5*A, so each path gets 0.5*s = s^3 — exactly the reference scaling.
"""

import sys

import numpy as np

if "/opt/trn_rl_repo" not in sys.path:
    sys.path.insert(0, "/opt/trn_rl_repo")

B, N, C = 4, 128, 4
N_CORES = 8
SLAB = 64          # d1 extent per core
D1C = 8            # d1 values per chunk
NCHUNK = SLAB // D1C
PAIRS = D1C // 2   # d1 pairs per chunk

_BASS_CACHE = {}

# bf16 matmul path: stationary weights are the exactly-representable +-0.5
# sign pattern of A; the residual 1/sqrt(2) is applied in the PSUM evacuation.
BF16_MM = False
EVAC_SCALE = float(1.0 / np.sqrt(2.0)) if BF16_MM else 1.0


def _haar_matrix():
    s = np.float32(1.0 / np.sqrt(2.0))
    A = np.zeros((N, N), dtype=np.float32)
    for i in range(N // 2):
        A[i, 2 * i] = s
        A[i, 2 * i + 1] = s
        A[64 + i, 2 * i] = -s
        A[64 + i, 2 * i + 1] = s
    return A


def _reference_numpy(inputs, A):
    # Fallback only: exact reference math on host (used if A is not Haar).
    x = np.einsum("ij,bpjqc->bpiqc", A, inputs)
    x = np.einsum("ij,bjpqc->bipqc", A, x)
    x = np.einsum("ij,bpqjc->bpqic", A, x)
    m = x.shape[1] // 2
    subs = [
        x[:, :m, :m, :m, :], x[:, :m, :m, m:, :],
        x[:, :m, m:, :m, :], x[:, :m, m:, m:, :],
        x[:, m:, :m, :m, :], x[:, m:, :m, m:, :],
        x[:, m:, m:, :m, :], x[:, m:, m:, m:, :],
    ]
    return np.concatenate(subs, axis=-1).astype(np.float32)


def _build_bass():
    import concourse.bacc as bacc
    import concourse.mybir as mybir
    import concourse.tile as tile

    f32 = mybir.dt.float32
    f32r = mybir.dt.float32r
    bf16 = mybir.dt.bfloat16
    mm_dt = bf16 if BF16_MM else f32r

    # Bacc (not raw Bass): its compile() pipeline splits multi-sem waits into
    # EventSemaphore instructions — TRN2 instructions have one wait slot.
    nc = bacc.Bacc("TRN2", target_bir_lowering=False, debug=False)
    # x is host-pre-transposed to [d2, d1, d3, c] so each load descriptor
    # covers a 16 KiB contiguous run per partition (descriptor generation on
    # the HWDGE sequencer costs ~4 ns/descriptor and was a bottleneck).
    x = nc.dram_tensor("x", [N, SLAB, N, C], f32, kind="ExternalInput")
    atp = nc.dram_tensor("atp", [N, N], mm_dt, kind="ExternalInput")
    atn = nc.dram_tensor("atn", [N, N], mm_dt, kind="ExternalInput")
    # y dims: (s1, s3, i2, o1, o3, c); i2 = s2*64 + o2. i2 outermost of the
    # spatial dims so each store descriptor is a 4 KiB contiguous run.
    y = nc.dram_tensor("y", [2, 2, N, 32, 64, C], f32, kind="ExternalOutput")

    with tile.TileContext(nc) as tc:
        with (
            tc.tile_pool(name="const", bufs=1) as cpool,
            tc.tile_pool(name="io", bufs=6) as tpool,
            tc.tile_pool(name="mid", bufs=3) as mpool,
            tc.tile_pool(name="psum", bufs=4, space="PSUM") as ppool,
        ):
            atp_sb = cpool.tile([N, N], mm_dt)
            atn_sb = cpool.tile([N, N], mm_dt)
            if not BF16_MM:
                # FP32r matmul operands must be produced pre-rounded to FP32r.
                atp_rt = cpool.tile([N, N], f32r)
                atn_rt = cpool.tile([N, N], f32r)
                atp_r = atp_rt[:]
                atn_r = atn_rt[:]
            else:
                atp_r = atp_sb[:]
                atn_r = atn_sb[:]

            for ci in range(NCHUNK):
                # 1. load chunk: [d2 | d1_local, d3*c] — one DMA,
                # 128 descriptors of 8 KiB.
                T = tpool.tile([N, D1C, N * C], f32, tag="T")
                nc.sync.dma_start(
                    out=T[:],
                    in_=x[:, ci * D1C:(ci + 1) * D1C].rearrange("p a q c -> p a (q c)"),
                )
                if ci == 0:
                    # consts after the first bulk load so the data pipeline
                    # starts immediately
                    nc.sync.dma_start(out=atp_sb[:], in_=atp[:, :])
                    nc.sync.dma_start(out=atn_sb[:], in_=atn[:, :])
                    if not BF16_MM:
                        nc.vector.tensor_copy(out=atp_rt[:], in_=atp_sb[:])
                        nc.vector.tensor_copy(out=atn_rt[:], in_=atn_sb[:])
                Tv = T[:].rearrange("p a (m t c) -> p a m t c", t=2, c=C)

                # 2. d3 butterfly: W[:, :, 0] = even+odd (low), [:, :, 1] = odd-even
                W = mpool.tile([N, D1C, 2, 64, C], mm_dt, tag="W")

                # staging: (s1, s3, o1_local, o3*c)
                Yst = mpool.tile([N, 2, 2, PAIRS, 64 * C], f32, tag="Yst")

                for pp in range(PAIRS):
                    # d3 butterfly per d1-pair so matmuls start as soon as
                    # their slice is ready (keeps the PE warm)
                    sl = slice(2 * pp, 2 * pp + 2)
                    nc.vector.tensor_add(
                        out=W[:, sl, 0], in0=Tv[:, sl, :, 0], in1=Tv[:, sl, :, 1]
                    )
                    nc.vector.tensor_sub(
                        out=W[:, sl, 1], in0=Tv[:, sl, :, 1], in1=Tv[:, sl, :, 0]
                    )
                    rhs0 = W[:, 2 * pp].rearrange("p k m c -> p (k m c)")
                    rhs1 = W[:, 2 * pp + 1].rearrange("p k m c -> p (k m c)")
                    ps_lo = ppool.tile([N, 512], f32, tag="pslo")
                    ps_hi = ppool.tile([N, 512], f32, tag="pshi")
                    # 3. d2 transform + d1 butterfly in PSUM
                    nc.tensor.matmul(ps_lo[:], lhsT=atp_r, rhs=rhs0, start=True, stop=False)
                    nc.tensor.matmul(ps_lo[:], lhsT=atp_r, rhs=rhs1, start=False, stop=True)
                    nc.tensor.matmul(ps_hi[:], lhsT=atp_r, rhs=rhs1, start=True, stop=False)
                    nc.tensor.matmul(ps_hi[:], lhsT=atn_r, rhs=rhs0, start=False, stop=True)
                    # 4. evacuate + subband split (s3 halves of free dim) on
                    # the scalar engine; applies the residual scale (1 for
                    # f32r weights, 1/sqrt(2) for the exact +-0.5 bf16 ones).
                    nc.scalar.mul(
                        Yst[:, 0, :, pp],
                        ps_lo[:].rearrange("p (k f) -> p k f", k=2),
                        EVAC_SCALE,
                    )
                    nc.scalar.mul(
                        Yst[:, 1, :, pp],
                        ps_hi[:].rearrange("p (k f) -> p k f", k=2),
                        EVAC_SCALE,
                    )

                # 5. store per (s1, s3): y[s1, s3, :, o1 range] <- [i2 | o1, o3*c]
                # SWDGE (gpsimd) so stores never head-of-line-block the load
                # queue on the SP sequencer while waiting for their copies.
                for s1 in range(2):
                    for s3 in range(2):
                        nc.gpsimd.dma_start(
                            out=y[s1, s3, :, ci * PAIRS:(ci + 1) * PAIRS].rearrange(
                                "p a q c -> p a (q c)"
                            ),
                            in_=Yst[:, s1, s3],
                        )
    nc.compile()
    return nc


def kernel(**inputs):
    x = np.ascontiguousarray(np.asarray(inputs["inputs"], dtype=np.float32))
    A = np.asarray(inputs["A"], dtype=np.float32)
    assert x.shape == (B, N, N, N, C), x.shape

    if not np.allclose(A, _haar_matrix(), atol=1e-5):
        # Kernel hardcodes the 2-tap Haar structure; fall back for generic A.
        return _reference_numpy(x, A)

    from concourse.bass_utils import run_bass_kernel_spmd

    if "nc" not in _BASS_CACHE:
        _BASS_CACHE["nc"] = _build_bass()
    nc = _BASS_CACHE["nc"]

    if BF16_MM:
        import ml_dtypes
        sign = np.sign(A.T).astype(np.float32)
        atp = np.ascontiguousarray((0.5 * sign).astype(ml_dtypes.bfloat16))
        atn = np.ascontiguousarray((-0.5 * sign).astype(ml_dtypes.bfloat16))
    else:
        atp = np.ascontiguousarray(0.5 * A.T)
        atn = np.ascontiguousarray(-atp)
    in_maps = []
    for k in range(N_CORES):
        b, h = divmod(k, 2)
        # pre-transpose slab to [d2, d1, d3, c] for contiguous load rows
        in_maps.append(
            {
                "x": np.ascontiguousarray(
                    x[b, h * SLAB:(h + 1) * SLAB].transpose(1, 0, 2, 3)
                ),
                "atp": atp,
                "atn": atn,
            }
        )

    res = run_bass_kernel_spmd(nc, in_maps, core_ids=list(range(N_CORES)))

    out = np.empty((B, 64, 64, 64, 8 * C), np.float32)
    for k in range(N_CORES):
        b, h = divmod(k, 2)
        arr = res.results[k]["y"].reshape(2, 2, 2, 64, 32, 64, C)
        # (s1, s3, s2, o2, o1, o3, c) -> (o1, o2, o3, s1, s2, s3, c)
        out[b, 32 * h:32 * h + 32] = (
            arr.transpose(4, 3, 5, 0, 2, 1, 6).reshape(32, 64, 64, 8 * C)
        )
    return out


# revision 27
# speedup vs baseline: 1.1829x; 1.1829x over previous
"""3D Haar DWT (clean-mode subband stack) on 8 Trainium2 NeuronCores.

Problem (hardcoded): inputs (4, 128, 128, 128, 4) f32, A (128, 128) f32 Haar
analysis operator. Output (4, 64, 64, 64, 32) f32 = 8 subbands stacked on the
channel axis (LLL, LLH, LHL, LHH, HLL, HLH, HHL, HHH) x 4 channels.

Sharding: pure data parallel over (batch, d1-half): core k handles
b = k // 2, d1 range [64*(k%2), 64*(k%2)+64). The Haar transform is a 2-tap
non-overlapping filter (rows of A touch only columns 2i, 2i+1), so splitting
d1 on an even boundary requires no communication.

Per-core pipeline (slab pre-transposed on host to [d2, d1, d3, c]):
  1. DMA in 2 MiB chunks (8 d1 slices), partitions = d2, 16 KiB descriptors.
  2. d3 butterfly on DVE (free-axis strided add/sub), per d1-pair.
  3. d2 transform as PE matmul (stationary 0.5*A^T, float32r fast path),
     with the d1 butterfly folded into PSUM accumulation (+/-0.5*A^T).
  4. PSUM -> SBUF evacuation on ACT doubles as the subband split.
  5. DMA out per (s1, s3) block on SWDGE (so stores never head-of-line-block
     the load queue); host reassembles the subband-major layout.

Scale bookkeeping: reference applies A three times (factor s = 1/sqrt(2) per
nonzero). Here the d3/d1 butterflies apply +/-1 and the matmul applies
0.5*A, so each path gets 0.5*s = s^3 — exactly the reference scaling.

Measured: ~94.3 us per core (8 cores in parallel), ~99% of the 358 GB/s
per-core HBM roofline for the 32 MiB (16 in + 16 out) of traffic.
"""

import sys

import numpy as np

if "/opt/trn_rl_repo" not in sys.path:
    sys.path.insert(0, "/opt/trn_rl_repo")

B, N, C = 4, 128, 4
N_CORES = 8
SLAB = 64          # d1 extent per core
D1C = 8            # d1 values per chunk
NCHUNK = SLAB // D1C
PAIRS = D1C // 2   # d1 pairs per chunk

_BASS_CACHE = {}

# bf16 matmul path: stationary weights are the exactly-representable +-0.5
# sign pattern of A; the residual 1/sqrt(2) is applied in the PSUM evacuation.
BF16_MM = False
EVAC_SCALE = float(1.0 / np.sqrt(2.0)) if BF16_MM else 1.0


def _haar_matrix():
    s = np.float32(1.0 / np.sqrt(2.0))
    A = np.zeros((N, N), dtype=np.float32)
    for i in range(N // 2):
        A[i, 2 * i] = s
        A[i, 2 * i + 1] = s
        A[64 + i, 2 * i] = -s
        A[64 + i, 2 * i + 1] = s
    return A


def _reference_numpy(inputs, A):
    # Fallback only: exact reference math on host (used if A is not Haar).
    x = np.einsum("ij,bpjqc->bpiqc", A, inputs)
    x = np.einsum("ij,bjpqc->bipqc", A, x)
    x = np.einsum("ij,bpqjc->bpqic", A, x)
    m = x.shape[1] // 2
    subs = [
        x[:, :m, :m, :m, :], x[:, :m, :m, m:, :],
        x[:, :m, m:, :m, :], x[:, :m, m:, m:, :],
        x[:, m:, :m, :m, :], x[:, m:, :m, m:, :],
        x[:, m:, m:, :m, :], x[:, m:, m:, m:, :],
    ]
    return np.concatenate(subs, axis=-1).astype(np.float32)


def _build_bass():
    import concourse.bacc as bacc
    import concourse.mybir as mybir
    import concourse.tile as tile

    f32 = mybir.dt.float32
    f32r = mybir.dt.float32r
    bf16 = mybir.dt.bfloat16
    mm_dt = bf16 if BF16_MM else f32r

    # Bacc (not raw Bass): its compile() pipeline splits multi-sem waits into
    # EventSemaphore instructions — TRN2 instructions have one wait slot.
    nc = bacc.Bacc("TRN2", target_bir_lowering=False, debug=False)
    # x is host-pre-transposed to [d2, d1, d3, c] so each load descriptor
    # covers a 16 KiB contiguous run per partition (descriptor generation on
    # the HWDGE sequencer costs ~4 ns/descriptor and was a bottleneck).
    x = nc.dram_tensor("x", [N, SLAB, N, C], f32, kind="ExternalInput")
    atp = nc.dram_tensor("atp", [N, N], mm_dt, kind="ExternalInput")
    atn = nc.dram_tensor("atn", [N, N], mm_dt, kind="ExternalInput")
    # y dims: (s1, s3, i2, o1, o3, c); i2 = s2*64 + o2. i2 outermost of the
    # spatial dims so each store descriptor is a 4 KiB contiguous run.
    y = nc.dram_tensor("y", [2, 2, N, 32, 64, C], f32, kind="ExternalOutput")

    with tile.TileContext(nc) as tc:
        with (
            tc.tile_pool(name="const", bufs=1) as cpool,
            tc.tile_pool(name="io", bufs=6) as tpool,
            tc.tile_pool(name="mid", bufs=3) as mpool,
            tc.tile_pool(name="psum", bufs=4, space="PSUM") as ppool,
        ):
            atp_sb = cpool.tile([N, N], mm_dt)
            atn_sb = cpool.tile([N, N], mm_dt)
            if not BF16_MM:
                # FP32r matmul operands must be produced pre-rounded to FP32r.
                atp_rt = cpool.tile([N, N], f32r)
                atn_rt = cpool.tile([N, N], f32r)
                atp_r = atp_rt[:]
                atn_r = atn_rt[:]
            else:
                atp_r = atp_sb[:]
                atn_r = atn_sb[:]

            for ci in range(NCHUNK):
                # 1. load chunk: [d2 | d1_local, d3*c] — one DMA,
                # 128 descriptors of 8 KiB.
                T = tpool.tile([N, D1C, N * C], f32, tag="T")
                nc.sync.dma_start(
                    out=T[:],
                    in_=x[:, ci * D1C:(ci + 1) * D1C].rearrange("p a q c -> p a (q c)"),
                )
                if ci == 0:
                    # consts after the first bulk load so the data pipeline
                    # starts immediately
                    nc.sync.dma_start(out=atp_sb[:], in_=atp[:, :])
                    nc.sync.dma_start(out=atn_sb[:], in_=atn[:, :])
                    if not BF16_MM:
                        nc.vector.tensor_copy(out=atp_rt[:], in_=atp_sb[:])
                        nc.vector.tensor_copy(out=atn_rt[:], in_=atn_sb[:])
                Tv = T[:].rearrange("p a (m t c) -> p a m t c", t=2, c=C)

                # 2. d3 butterfly: W[:, :, 0] = even+odd (low), [:, :, 1] = odd-even
                W = mpool.tile([N, D1C, 2, 64, C], mm_dt, tag="W")

                # staging: (s1, s3, o1_local, o3*c)
                Yst = mpool.tile([N, 2, 2, PAIRS, 64 * C], f32, tag="Yst")

                for pp in range(PAIRS):
                    # d3 butterfly per d1-pair so matmuls start as soon as
                    # their slice is ready (keeps the PE warm)
                    sl = slice(2 * pp, 2 * pp + 2)
                    nc.vector.tensor_add(
                        out=W[:, sl, 0], in0=Tv[:, sl, :, 0], in1=Tv[:, sl, :, 1]
                    )
                    nc.vector.tensor_sub(
                        out=W[:, sl, 1], in0=Tv[:, sl, :, 1], in1=Tv[:, sl, :, 0]
                    )
                    rhs0 = W[:, 2 * pp].rearrange("p k m c -> p (k m c)")
                    rhs1 = W[:, 2 * pp + 1].rearrange("p k m c -> p (k m c)")
                    ps_lo = ppool.tile([N, 512], f32, tag="pslo")
                    ps_hi = ppool.tile([N, 512], f32, tag="pshi")
                    # 3. d2 transform + d1 butterfly in PSUM
                    nc.tensor.matmul(ps_lo[:], lhsT=atp_r, rhs=rhs0, start=True, stop=False)
                    nc.tensor.matmul(ps_lo[:], lhsT=atp_r, rhs=rhs1, start=False, stop=True)
                    nc.tensor.matmul(ps_hi[:], lhsT=atp_r, rhs=rhs1, start=True, stop=False)
                    nc.tensor.matmul(ps_hi[:], lhsT=atn_r, rhs=rhs0, start=False, stop=True)
                    # 4. evacuate + subband split (s3 halves of free dim) on
                    # the scalar engine; applies the residual scale (1 for
                    # f32r weights, 1/sqrt(2) for the exact +-0.5 bf16 ones).
                    nc.scalar.mul(
                        Yst[:, 0, :, pp],
                        ps_lo[:].rearrange("p (k f) -> p k f", k=2),
                        EVAC_SCALE,
                    )
                    nc.scalar.mul(
                        Yst[:, 1, :, pp],
                        ps_hi[:].rearrange("p (k f) -> p k f", k=2),
                        EVAC_SCALE,
                    )

                # 5. store per (s1, s3): y[s1, s3, :, o1 range] <- [i2 | o1, o3*c]
                # SWDGE (gpsimd) so stores never head-of-line-block the load
                # queue on the SP sequencer while waiting for their copies.
                for s1 in range(2):
                    for s3 in range(2):
                        nc.gpsimd.dma_start(
                            out=y[s1, s3, :, ci * PAIRS:(ci + 1) * PAIRS].rearrange(
                                "p a q c -> p a (q c)"
                            ),
                            in_=Yst[:, s1, s3],
                        )
    nc.compile()
    return nc


def kernel(**inputs):
    x = np.ascontiguousarray(np.asarray(inputs["inputs"], dtype=np.float32))
    A = np.asarray(inputs["A"], dtype=np.float32)
    assert x.shape == (B, N, N, N, C), x.shape

    if not np.allclose(A, _haar_matrix(), atol=1e-5):
        # Kernel hardcodes the 2-tap Haar structure; fall back for generic A.
        return _reference_numpy(x, A)

    from concourse.bass_utils import run_bass_kernel_spmd

    if "nc" not in _BASS_CACHE:
        _BASS_CACHE["nc"] = _build_bass()
    nc = _BASS_CACHE["nc"]

    if BF16_MM:
        import ml_dtypes
        sign = np.sign(A.T).astype(np.float32)
        atp = np.ascontiguousarray((0.5 * sign).astype(ml_dtypes.bfloat16))
        atn = np.ascontiguousarray((-0.5 * sign).astype(ml_dtypes.bfloat16))
    else:
        atp = np.ascontiguousarray(0.5 * A.T)
        atn = np.ascontiguousarray(-atp)
    in_maps = []
    for k in range(N_CORES):
        b, h = divmod(k, 2)
        # pre-transpose slab to [d2, d1, d3, c] for contiguous load rows
        in_maps.append(
            {
                "x": np.ascontiguousarray(
                    x[b, h * SLAB:(h + 1) * SLAB].transpose(1, 0, 2, 3)
                ),
                "atp": atp,
                "atn": atn,
            }
        )

    res = run_bass_kernel_spmd(nc, in_maps, core_ids=list(range(N_CORES)))

    out = np.empty((B, 64, 64, 64, 8 * C), np.float32)
    for k in range(N_CORES):
        b, h = divmod(k, 2)
        arr = res.results[k]["y"].reshape(2, 2, 2, 64, 32, 64, C)
        # (s1, s3, s2, o2, o1, o3, c) -> (o1, o2, o3, s1, s2, s3, c)
        out[b, 32 * h:32 * h + 32] = (
            arr.transpose(4, 3, 5, 0, 2, 1, 6).reshape(32, 64, 64, 8 * C)
        )
    return out
